# revision 2
# baseline (speedup 1.0000x reference)
"""Trainium2 Bass kernel for nn_ConvSPE (depthwise-conv SPE + per-channel contraction).

Math (reference): per bn=(b,nu) row and channel d:
    pe_k = noise / sqrt(num*d)                       (b*num, d, s+2k)
    pe_q = depthwise_valid_xcorr(pe_k, w)            k=200 taps, same filter per channel
    qhat[b,nu,t] = sum_d pe_q[bn,d,t]      * q[b,d,t]
    khat[b,nu,t] = sum_d pe_k[bn,d,t+k//2] * k[b,d,t]

Kernel strategy (8 NeuronCores, data-parallel over the 128 bn rows; 16 rows/core):
  * Host pre-arranges noise into a time-partition-inner fp16 layout
    xf[bn][p, n, d] = noise[bn, d, 128n+p] so the conv becomes 3 PSUM-accumulated
    TensorE matmuls per output block with fixed Toeplitz weights
    W_s[p, m] = w[p + 128s - m] (shared across all channels/rows).
  * qhat: multiply conv PSUM by host-pre-transposed queries on VectorE, then
    free-axis reduce over d.
  * khat needs no conv: multiply xf by a host-shifted/scaled keys layout
    (shift of k//2=100 and the 1/sqrt(num*d) scale are baked in), reduce over d.
"""

import math
import numpy as np

_CACHE = {}


def _ensure_paths():
    try:
        import concourse  # noqa: F401
    except ImportError:
        import sys

        for p in ("/opt/trn_rl_repo", "/root/.axon_site/_ro/trn_rl_repo"):
            if p not in sys.path:
                sys.path.insert(0, p)


N_CORES = 8
B, D, L, K, NUM = 4, 64, 4096, 200, 32
NW = 34  # x windows of 128 loaded per row (covers t+j up to 4351)
NT = 32  # output time blocks of 128
NK = 33  # khat product blocks (u = t + 100 spans [0, 4224))
ROWS = 16  # bn rows per core


def build_module():
    """Build + compile the per-core Bass module (identical SPMD program)."""
    _ensure_paths()
    from contextlib import ExitStack

    import concourse.bacc as bacc
    import concourse.mybir as mybir
    import concourse.tile as tile

    F16 = mybir.dt.float16
    F32 = mybir.dt.float32
    X = mybir.AxisListType.X

    nc = bacc.Bacc(
        "TRN2", target_bir_lowering=False, debug=False, num_devices=N_CORES
    )

    xf_d = nc.dram_tensor("xf", [ROWS, 128, NW, D], F16, kind="ExternalInput").ap()
    wq_d = nc.dram_tensor("wq", [3, 128, 128], F16, kind="ExternalInput").ap()
    qt_d = nc.dram_tensor("qt", [128, NT, D], F32, kind="ExternalInput").ap()
    kf_d = nc.dram_tensor("kf", [128, NK, D], F16, kind="ExternalInput").ap()
    qo_d = nc.dram_tensor("qo", [128, ROWS, NT], F32, kind="ExternalOutput").ap()
    ko_d = nc.dram_tensor("ko", [128, ROWS, NK], F32, kind="ExternalOutput").ap()

    with tile.TileContext(nc) as tc, ExitStack() as ctx:
        wp = ctx.enter_context(tc.tile_pool(name="const", bufs=1))
        xp = ctx.enter_context(tc.tile_pool(name="x", bufs=3))
        pp = ctx.enter_context(tc.tile_pool(name="psum", bufs=8, space="PSUM"))
        qp = ctx.enter_context(tc.tile_pool(name="prodq", bufs=4))
        kpool = ctx.enter_context(tc.tile_pool(name="prodk", bufs=2))
        op = ctx.enter_context(tc.tile_pool(name="out", bufs=1))

        wts = []
        for s in range(3):
            t = wp.tile([128, 128], F16, tag=f"w{s}")
            nc.sync.dma_start(t[:], wq_d[s])
            wts.append(t)
        qt_t = wp.tile([128, NT, D], F32, tag="qt")
        nc.sync.dma_start(qt_t[:], qt_d[:])
        kf_t = wp.tile([128, NK, D], F16, tag="kf")
        nc.sync.dma_start(kf_t[:], kf_d[:])

        qacc = op.tile([128, ROWS, NT], F32, tag="qa")
        kacc = op.tile([128, ROWS, NK], F32, tag="ka")

        for bn in range(ROWS):
            xt = xp.tile([128, NW, D], F16, tag="xt")
            nc.sync.dma_start(xt[:], xf_d[bn])

            # ---- qhat path: conv via 3 Toeplitz matmuls per 8-block group
            pss = []
            for g in range(4):
                pss.append(pp.tile([128, 8, D], F32, tag="ps", name=f"ps_{bn}_{g}"))
            for s in range(3):
                for g in range(4):
                    nc.tensor.matmul(
                        pss[g][:],
                        wts[s][:],
                        xt[:, g * 8 + s : g * 8 + s + 8, :],
                        start=(s == 0),
                        stop=(s == 2),
                    )
            for g in range(4):
                pq = qp.tile([128, 8, D], F16, tag="pq")
                nc.vector.tensor_mul(pq[:], pss[g][:], qt_t[:, g * 8 : (g + 1) * 8, :])
                nc.vector.reduce_sum(
                    qacc[:, bn, g * 8 : (g + 1) * 8], pq[:], axis=X
                )

            # ---- khat path: pure elementwise + reduce
            pk = kpool.tile([128, NK, D], F16, tag="pk")
            nc.vector.tensor_mul(pk[:], xt[:, 0:NK, :], kf_t[:])
            nc.vector.reduce_sum(kacc[:, bn, :], pk[:], axis=X)

        nc.sync.dma_start(qo_d[:], qacc[:])
        nc.sync.dma_start(ko_d[:], kacc[:])

    nc.compile()
    return nc


def _get_module():
    if "nc" not in _CACHE:
        _CACHE["nc"] = build_module()
    return _CACHE["nc"]


def make_in_maps(queries, keys, noise, conv_weight, num):
    """Host-side shard + re-layout (all cheap numpy ops)."""
    num = int(np.asarray(num))
    queries = np.asarray(queries, dtype=np.float32)
    keys = np.asarray(keys, dtype=np.float32)
    noise = np.asarray(noise, dtype=np.float32)
    w = np.asarray(conv_weight, dtype=np.float32)[0, 0, :]
    scale = 1.0 / math.sqrt(num * D)

    # Toeplitz weights (scale folded in): W_s[p, m] = w[p + 128s - m] * scale
    p = np.arange(128)[:, None]
    m = np.arange(128)[None, :]
    Wq = np.zeros((3, 128, 128), np.float32)
    for s in range(3):
        j = p + 128 * s - m
        mask = (j >= 0) & (j < K)
        Wq[s][mask] = w[j[mask]] * scale
    Wq16 = Wq.astype(np.float16)

    # xf[bn][p, n, d] = noise[bn, d, 128n + p]
    xf = (
        noise[:, :, : NW * 128]
        .reshape(B * NUM, D, NW, 128)
        .transpose(0, 3, 2, 1)
        .astype(np.float16)
    )
    # qt[b][p, tau, d] = queries[b, d, 128 tau + p]
    qt = queries.reshape(B, D, NT, 128).transpose(0, 3, 2, 1).astype(np.float32)
    # kf[b][p, n, d] = keys[b, d, 128n + p - 100] * scale (zero out of range)
    kp = np.zeros((B, D, NK * 128), np.float32)
    kp[:, :, K // 2 : K // 2 + L] = keys * scale
    kf = kp.reshape(B, D, NK, 128).transpose(0, 3, 2, 1).astype(np.float16)

    in_maps = []
    for c in range(N_CORES):
        b = (c * ROWS) // NUM // 1  # 16 rows/core, 32 rows/b -> b = c // 2
        b = c // 2
        in_maps.append(
            {
                "xf": np.ascontiguousarray(xf[ROWS * c : ROWS * (c + 1)]),
                "wq": Wq16,
                "qt": np.ascontiguousarray(qt[b]),
                "kf": np.ascontiguousarray(kf[b]),
            }
        )
    return in_maps


def assemble_outputs(results):
    qhat = np.empty((B * NUM, L), np.float32)
    khat = np.empty((B * NUM, L), np.float32)
    for c in range(N_CORES):
        qo = results[c]["qo"]  # [128, ROWS, NT]
        ko = results[c]["ko"]  # [128, ROWS, NK]
        qhat[ROWS * c : ROWS * (c + 1)] = qo.transpose(1, 2, 0).reshape(ROWS, L)
        kv = ko.transpose(1, 2, 0).reshape(ROWS, NK * 128)
        khat[ROWS * c : ROWS * (c + 1)] = kv[:, K // 2 : K // 2 + L]
    return (
        qhat.reshape(B, NUM, L),
        khat.reshape(B, NUM, L),
    )


def kernel(queries, keys, noise, conv_weight, num):
    _ensure_paths()
    from concourse import bass_utils

    in_maps = make_in_maps(queries, keys, noise, conv_weight, num)
    nc = _get_module()
    res = bass_utils.run_bass_kernel_spmd(nc, in_maps, core_ids=list(range(N_CORES)))
    return assemble_outputs(res.results)


# revision 3
# speedup vs baseline: 1.3343x; 1.3343x over previous
"""Trainium2 Bass kernel for nn_ConvSPE (depthwise-conv SPE + per-channel contraction).

Math (reference): per bn=(b,nu) row and channel d:
    pe_k = noise / sqrt(num*d)                       (b*num, d, s+2k)
    pe_q = depthwise_valid_xcorr(pe_k, w)            k=200 taps, same filter per channel
    qhat[b,nu,t] = sum_d pe_q[bn,d,t]      * q[b,d,t]
    khat[b,nu,t] = sum_d pe_k[bn,d,t+k//2] * k[b,d,t]

Kernel strategy (8 NeuronCores, data-parallel over the 128 bn rows; 16 rows/core):
  * Host pre-arranges noise into a time-partition-inner fp16 layout
    xf[bn][p, n, d] = noise[bn, d, 128n+p] so the conv becomes 3 PSUM-accumulated
    TensorE matmuls per output block with fixed Toeplitz weights
    W_s[p, m] = w[p + 128s - m] (shared across all channels/rows).
  * qhat: ScalarE copies conv PSUM -> SBUF fp16, VectorE multiplies by
    host-pre-transposed queries (fp16 2x mode) and reduces over d.
  * khat needs no conv: VectorE multiplies xf by a host-shifted/scaled keys
    layout (shift k//2=100 and 1/sqrt(num*d) baked in); GpSimd reduces over d
    with an fp32 add-tree (engine balancing: DVE is the bottleneck).
"""

import math
import numpy as np

_CACHE = {}


def _ensure_paths():
    try:
        import concourse  # noqa: F401
    except ImportError:
        import sys

        for p in ("/opt/trn_rl_repo", "/root/.axon_site/_ro/trn_rl_repo"):
            if p not in sys.path:
                sys.path.insert(0, p)


N_CORES = 8
B, D, L, K, NUM = 4, 64, 4096, 200, 32
NW = 34  # x windows of 128 loaded per row (covers t+j up to 4351)
NT = 32  # output time blocks of 128
NK = 33  # khat product blocks (u = t + 100 spans [0, 4224))
ROWS = 16  # bn rows per core


def _add_tree(eng, pool, acc_out, src, n_outer, bn, mybir):
    """Reduce src [128, n_outer, 64] over the last axis into acc_out [128, n_outer]
    using TT-adds (fp32 after level 1). Works on engines without X-axis reduce."""
    F32 = mybir.dt.float32
    a = pool.tile([128, n_outer, 32], F32, tag="treeA", name=f"treeA_{bn}")
    b = pool.tile([128, n_outer, 16], F32, tag="treeB", name=f"treeB_{bn}")
    eng.tensor_add(a[:], src[:, :, 0:32], src[:, :, 32:64])
    eng.tensor_add(b[:], a[:, :, 0:16], a[:, :, 16:32])
    eng.tensor_add(a[:, :, 0:8], b[:, :, 0:8], b[:, :, 8:16])
    eng.tensor_add(b[:, :, 0:4], a[:, :, 0:4], a[:, :, 4:8])
    eng.tensor_add(a[:, :, 8:10], b[:, :, 0:2], b[:, :, 2:4])
    eng.tensor_add(acc_out, a[:, :, 8], a[:, :, 9])


def build_module():
    """Build + compile the per-core Bass module (identical SPMD program)."""
    _ensure_paths()
    from contextlib import ExitStack

    import concourse.bacc as bacc
    import concourse.mybir as mybir
    import concourse.tile as tile

    F16 = mybir.dt.float16
    F32 = mybir.dt.float32
    X = mybir.AxisListType.X

    nc = bacc.Bacc(
        "TRN2", target_bir_lowering=False, debug=False, num_devices=N_CORES
    )

    xf_d = nc.dram_tensor("xf", [ROWS, 128, NW, D], F16, kind="ExternalInput").ap()
    wq_d = nc.dram_tensor("wq", [3, 128, 128], F16, kind="ExternalInput").ap()
    qt_d = nc.dram_tensor("qt", [128, NT, D], F16, kind="ExternalInput").ap()
    kf_d = nc.dram_tensor("kf", [128, NK, D], F16, kind="ExternalInput").ap()
    qo_d = nc.dram_tensor("qo", [128, ROWS, NT], F32, kind="ExternalOutput").ap()
    ko_d = nc.dram_tensor("ko", [128, ROWS, NK], F32, kind="ExternalOutput").ap()

    with tile.TileContext(nc) as tc, ExitStack() as ctx:
        wp = ctx.enter_context(tc.tile_pool(name="const", bufs=1))
        xp = ctx.enter_context(tc.tile_pool(name="x", bufs=3))
        pp = ctx.enter_context(tc.tile_pool(name="psum", bufs=2, space="PSUM"))
        cp = ctx.enter_context(tc.tile_pool(name="peq", bufs=2))
        qp = ctx.enter_context(tc.tile_pool(name="prodq", bufs=2))
        kpool = ctx.enter_context(tc.tile_pool(name="prodk", bufs=2))
        tp = ctx.enter_context(tc.tile_pool(name="tree", bufs=2))
        op = ctx.enter_context(tc.tile_pool(name="out", bufs=1))

        wts = []
        for s in range(3):
            t = wp.tile([128, 128], F16, tag=f"w{s}")
            nc.sync.dma_start(t[:], wq_d[s])
            wts.append(t)
        qt_t = wp.tile([128, NT, D], F16, tag="qt")
        nc.sync.dma_start(qt_t[:], qt_d[:])
        kf_t = wp.tile([128, NK, D], F16, tag="kf")
        nc.sync.dma_start(kf_t[:], kf_d[:])

        qacc = op.tile([128, ROWS, NT], F32, tag="qa")
        kacc = op.tile([128, ROWS, NK], F32, tag="ka")

        for bn in range(ROWS):
            xt = xp.tile([128, NW, D], F16, tag="xt", name=f"xt_{bn}")
            nc.sync.dma_start(xt[:], xf_d[bn])

            # ---- qhat path: conv via 3 Toeplitz matmuls per 8-block group,
            # one 4-bank PSUM tile per row.
            ps = pp.tile([128, NT, D], F32, tag="ps", name=f"ps_{bn}")
            for s in range(3):
                for g in range(4):
                    nc.tensor.matmul(
                        ps[:, g * 8 : (g + 1) * 8, :],
                        wts[s][:],
                        xt[:, g * 8 + s : g * 8 + s + 8, :],
                        start=(s == 0),
                        stop=(s == 2),
                    )
            peq = cp.tile([128, NT, D], F16, tag="peq", name=f"peq_{bn}")
            nc.scalar.copy(peq[:], ps[:])
            pq = qp.tile([128, NT, D], F16, tag="pq", name=f"pq_{bn}")
            nc.vector.tensor_mul(pq[:], peq[:], qt_t[:])
            nc.vector.reduce_sum(qacc[:, bn, :], pq[:], axis=X)

            # ---- khat path: pure elementwise + gpsimd reduce tree
            pk = kpool.tile([128, NK, D], F16, tag="pk", name=f"pk_{bn}")
            nc.vector.tensor_mul(pk[:], xt[:, 0:NK, :], kf_t[:])
            _add_tree(nc.gpsimd, tp, kacc[:, bn, :], pk, NK, bn, mybir)

        nc.sync.dma_start(qo_d[:], qacc[:])
        nc.sync.dma_start(ko_d[:], kacc[:])

    nc.compile()
    return nc


def _get_module():
    if "nc" not in _CACHE:
        _CACHE["nc"] = build_module()
    return _CACHE["nc"]


def make_in_maps(queries, keys, noise, conv_weight, num):
    """Host-side shard + re-layout (all cheap numpy ops)."""
    num = int(np.asarray(num))
    queries = np.asarray(queries, dtype=np.float32)
    keys = np.asarray(keys, dtype=np.float32)
    noise = np.asarray(noise, dtype=np.float32)
    w = np.asarray(conv_weight, dtype=np.float32)[0, 0, :]
    scale = 1.0 / math.sqrt(num * D)

    # Toeplitz weights (scale folded in): W_s[p, m] = w[p + 128s - m] * scale
    p = np.arange(128)[:, None]
    m = np.arange(128)[None, :]
    Wq = np.zeros((3, 128, 128), np.float32)
    for s in range(3):
        j = p + 128 * s - m
        mask = (j >= 0) & (j < K)
        Wq[s][mask] = w[j[mask]] * scale
    Wq16 = Wq.astype(np.float16)

    # xf[bn][p, n, d] = noise[bn, d, 128n + p]
    xf = (
        noise[:, :, : NW * 128]
        .reshape(B * NUM, D, NW, 128)
        .transpose(0, 3, 2, 1)
        .astype(np.float16)
    )
    # qt[b][p, tau, d] = queries[b, d, 128 tau + p]
    qt = queries.reshape(B, D, NT, 128).transpose(0, 3, 2, 1).astype(np.float16)
    # kf[b][p, n, d] = keys[b, d, 128n + p - 100] * scale (zero out of range)
    kp = np.zeros((B, D, NK * 128), np.float32)
    kp[:, :, K // 2 : K // 2 + L] = keys * scale
    kf = kp.reshape(B, D, NK, 128).transpose(0, 3, 2, 1).astype(np.float16)

    in_maps = []
    for c in range(N_CORES):
        b = c // 2
        in_maps.append(
            {
                "xf": np.ascontiguousarray(xf[ROWS * c : ROWS * (c + 1)]),
                "wq": Wq16,
                "qt": np.ascontiguousarray(qt[b]),
                "kf": np.ascontiguousarray(kf[b]),
            }
        )
    return in_maps


def assemble_outputs(results):
    qhat = np.empty((B * NUM, L), np.float32)
    khat = np.empty((B * NUM, L), np.float32)
    for c in range(N_CORES):
        qo = results[c]["qo"]  # [128, ROWS, NT]
        ko = results[c]["ko"]  # [128, ROWS, NK]
        qhat[ROWS * c : ROWS * (c + 1)] = qo.transpose(1, 2, 0).reshape(ROWS, L)
        kv = ko.transpose(1, 2, 0).reshape(ROWS, NK * 128)
        khat[ROWS * c : ROWS * (c + 1)] = kv[:, K // 2 : K // 2 + L]
    return (
        qhat.reshape(B, NUM, L),
        khat.reshape(B, NUM, L),
    )


def kernel(queries, keys, noise, conv_weight, num):
    _ensure_paths()
    from concourse import bass_utils

    in_maps = make_in_maps(queries, keys, noise, conv_weight, num)
    nc = _get_module()
    res = bass_utils.run_bass_kernel_spmd(nc, in_maps, core_ids=list(range(N_CORES)))
    return assemble_outputs(res.results)


# revision 4
# speedup vs baseline: 1.5100x; 1.1317x over previous
"""Trainium2 Bass kernel for nn_ConvSPE (depthwise-conv SPE + per-channel contraction).

Math (reference): per bn=(b,nu) row and channel d:
    pe_k = noise / sqrt(num*d)                       (b*num, d, s+2k)
    pe_q = depthwise_valid_xcorr(pe_k, w)            k=200 taps, same filter per channel
    qhat[b,nu,t] = sum_d pe_q[bn,d,t]      * q[b,d,t]
    khat[b,nu,t] = sum_d pe_k[bn,d,t+k//2] * k[b,d,t]

Kernel strategy (8 NeuronCores, data-parallel over the 128 bn rows; 16 rows/core):
  * Host pre-arranges noise into a time-partition-inner fp16 layout
    xf[bn][p, n, d] = noise[bn, d, 128n+p] so the conv becomes 3 PSUM-accumulated
    TensorE matmuls per output block with fixed Toeplitz weights
    W_s[p, m] = w[p + 128s - m] (shared across all channels/rows).
  * qhat: ScalarE copies conv PSUM -> SBUF fp16, VectorE multiplies by
    host-pre-transposed queries (fp16 2x mode) and reduces over d.
  * khat needs no conv: VectorE multiplies xf by a host-shifted/scaled keys
    layout (shift k//2=100 and 1/sqrt(num*d) baked in); GpSimd reduces over d
    with an fp32 add-tree (engine balancing: DVE is the bottleneck).
"""

import math
import numpy as np

_CACHE = {}


def _ensure_paths():
    try:
        import concourse  # noqa: F401
    except ImportError:
        import sys

        for p in ("/opt/trn_rl_repo", "/root/.axon_site/_ro/trn_rl_repo"):
            if p not in sys.path:
                sys.path.insert(0, p)


N_CORES = 8
B, D, L, K, NUM = 4, 64, 4096, 200, 32
NW = 34  # x windows of 128 loaded per row (covers t+j up to 4351)
NT = 32  # output time blocks of 128
NK = 33  # khat product blocks (u = t + 100 spans [0, 4224))
ROWS = 16  # bn rows per core


def _add_tree(eng, pool, acc_out, src, n_outer, bn, mybir):
    """Reduce src [128, n_outer, 64] over the last axis into acc_out [128, n_outer]
    using TT-adds (fp32 after level 1). Works on engines without X-axis reduce."""
    F32 = mybir.dt.float32
    a = pool.tile([128, n_outer, 32], F32, tag="treeA", name=f"treeA_{bn}")
    b = pool.tile([128, n_outer, 16], F32, tag="treeB", name=f"treeB_{bn}")
    eng.tensor_add(a[:], src[:, :, 0:32], src[:, :, 32:64])
    eng.tensor_add(b[:], a[:, :, 0:16], a[:, :, 16:32])
    eng.tensor_add(a[:, :, 0:8], b[:, :, 0:8], b[:, :, 8:16])
    eng.tensor_add(b[:, :, 0:4], a[:, :, 0:4], a[:, :, 4:8])
    eng.tensor_add(a[:, :, 8:10], b[:, :, 0:2], b[:, :, 2:4])
    eng.tensor_add(acc_out, a[:, :, 8], a[:, :, 9])


def build_module():
    """Build + compile the per-core Bass module (identical SPMD program)."""
    _ensure_paths()
    from contextlib import ExitStack

    import concourse.bacc as bacc
    import concourse.mybir as mybir
    import concourse.tile as tile

    F16 = mybir.dt.float16
    F32 = mybir.dt.float32
    X = mybir.AxisListType.X

    nc = bacc.Bacc(
        "TRN2", target_bir_lowering=False, debug=False, num_devices=N_CORES
    )

    xf_d = nc.dram_tensor("xf", [ROWS, 128, NW, D], F16, kind="ExternalInput").ap()
    wq_d = nc.dram_tensor("wq", [3, 128, 128], F16, kind="ExternalInput").ap()
    qt_d = nc.dram_tensor("qt", [128, NT, D], F16, kind="ExternalInput").ap()
    kf_d = nc.dram_tensor("kf", [128, NK, D], F16, kind="ExternalInput").ap()
    qo_d = nc.dram_tensor("qo", [128, ROWS, NT], F32, kind="ExternalOutput").ap()
    ko_d = nc.dram_tensor("ko", [128, ROWS, NK], F32, kind="ExternalOutput").ap()

    with tile.TileContext(nc) as tc, ExitStack() as ctx:
        wp = ctx.enter_context(tc.tile_pool(name="const", bufs=1))
        xp = ctx.enter_context(tc.tile_pool(name="x", bufs=4))
        pp = ctx.enter_context(tc.tile_pool(name="psum", bufs=2, space="PSUM"))
        cp = ctx.enter_context(tc.tile_pool(name="peq", bufs=3))
        qp = ctx.enter_context(tc.tile_pool(name="prodq", bufs=3))
        kpool = ctx.enter_context(tc.tile_pool(name="prodk", bufs=3))
        tp = ctx.enter_context(tc.tile_pool(name="tree", bufs=3))
        op = ctx.enter_context(tc.tile_pool(name="out", bufs=1))

        wts = []
        for s in range(3):
            t = wp.tile([128, 128], F16, tag=f"w{s}")
            nc.sync.dma_start(t[:], wq_d[s])
            wts.append(t)
        qt_t = wp.tile([128, NT, D], F16, tag="qt")
        nc.sync.dma_start(qt_t[:], qt_d[:])
        kf_t = wp.tile([128, NK, D], F16, tag="kf")
        nc.sync.dma_start(kf_t[:], kf_d[:])

        qacc = op.tile([128, ROWS, NT], F32, tag="qa")
        kacc = op.tile([128, ROWS, NK], F32, tag="ka")

        for bn in range(ROWS):
            xt = xp.tile([128, NW, D], F16, tag="xt", name=f"xt_{bn}")
            nc.sync.dma_start(xt[:], xf_d[bn])

            # ---- khat path: pure elementwise + gpsimd reduce tree
            pk = kpool.tile([128, NK, D], F16, tag="pk", name=f"pk_{bn}")
            nc.vector.tensor_mul(pk[:], xt[:, 0:NK, :], kf_t[:])
            _add_tree(nc.gpsimd, tp, kacc[:, bn, :], pk, NK, bn, mybir)

            # ---- qhat path: conv via 3 Toeplitz matmuls per 8-block group,
            # one 4-bank PSUM tile per row.
            ps = pp.tile([128, NT, D], F32, tag="ps", name=f"ps_{bn}")
            for s in range(3):
                for g in range(4):
                    nc.tensor.matmul(
                        ps[:, g * 8 : (g + 1) * 8, :],
                        wts[s][:],
                        xt[:, g * 8 + s : g * 8 + s + 8, :],
                        start=(s == 0),
                        stop=(s == 2),
                    )
            peq = cp.tile([128, NT, D], F16, tag="peq", name=f"peq_{bn}")
            nc.scalar.copy(peq[:], ps[:])
            pq = qp.tile([128, NT, D], F16, tag="pq", name=f"pq_{bn}")
            nc.vector.tensor_mul(pq[:], peq[:], qt_t[:])
            nc.vector.reduce_sum(qacc[:, bn, :], pq[:], axis=X)

        nc.sync.dma_start(qo_d[:], qacc[:])
        nc.sync.dma_start(ko_d[:], kacc[:])

    nc.compile()
    return nc


def _get_module():
    if "nc" not in _CACHE:
        _CACHE["nc"] = build_module()
    return _CACHE["nc"]


def make_in_maps(queries, keys, noise, conv_weight, num):
    """Host-side shard + re-layout (all cheap numpy ops)."""
    num = int(np.asarray(num))
    queries = np.asarray(queries, dtype=np.float32)
    keys = np.asarray(keys, dtype=np.float32)
    noise = np.asarray(noise, dtype=np.float32)
    w = np.asarray(conv_weight, dtype=np.float32)[0, 0, :]
    scale = 1.0 / math.sqrt(num * D)

    # Toeplitz weights (scale folded in): W_s[p, m] = w[p + 128s - m] * scale
    p = np.arange(128)[:, None]
    m = np.arange(128)[None, :]
    Wq = np.zeros((3, 128, 128), np.float32)
    for s in range(3):
        j = p + 128 * s - m
        mask = (j >= 0) & (j < K)
        Wq[s][mask] = w[j[mask]] * scale
    Wq16 = Wq.astype(np.float16)

    # xf[bn][p, n, d] = noise[bn, d, 128n + p]
    xf = (
        noise[:, :, : NW * 128]
        .reshape(B * NUM, D, NW, 128)
        .transpose(0, 3, 2, 1)
        .astype(np.float16)
    )
    # qt[b][p, tau, d] = queries[b, d, 128 tau + p]
    qt = queries.reshape(B, D, NT, 128).transpose(0, 3, 2, 1).astype(np.float16)
    # kf[b][p, n, d] = keys[b, d, 128n + p - 100] * scale (zero out of range)
    kp = np.zeros((B, D, NK * 128), np.float32)
    kp[:, :, K // 2 : K // 2 + L] = keys * scale
    kf = kp.reshape(B, D, NK, 128).transpose(0, 3, 2, 1).astype(np.float16)

    in_maps = []
    for c in range(N_CORES):
        b = c // 2
        in_maps.append(
            {
                "xf": np.ascontiguousarray(xf[ROWS * c : ROWS * (c + 1)]),
                "wq": Wq16,
                "qt": np.ascontiguousarray(qt[b]),
                "kf": np.ascontiguousarray(kf[b]),
            }
        )
    return in_maps


def assemble_outputs(results):
    qhat = np.empty((B * NUM, L), np.float32)
    khat = np.empty((B * NUM, L), np.float32)
    for c in range(N_CORES):
        qo = results[c]["qo"]  # [128, ROWS, NT]
        ko = results[c]["ko"]  # [128, ROWS, NK]
        qhat[ROWS * c : ROWS * (c + 1)] = qo.transpose(1, 2, 0).reshape(ROWS, L)
        kv = ko.transpose(1, 2, 0).reshape(ROWS, NK * 128)
        khat[ROWS * c : ROWS * (c + 1)] = kv[:, K // 2 : K // 2 + L]
    return (
        qhat.reshape(B, NUM, L),
        khat.reshape(B, NUM, L),
    )


def kernel(queries, keys, noise, conv_weight, num):
    _ensure_paths()
    from concourse import bass_utils

    in_maps = make_in_maps(queries, keys, noise, conv_weight, num)
    nc = _get_module()
    res = bass_utils.run_bass_kernel_spmd(nc, in_maps, core_ids=list(range(N_CORES)))
    return assemble_outputs(res.results)


# revision 5
# speedup vs baseline: 1.5213x; 1.0075x over previous
"""Trainium2 Bass kernel for nn_ConvSPE (depthwise-conv SPE + per-channel contraction).

Math (reference): per bn=(b,nu) row and channel d:
    pe_k = noise / sqrt(num*d)                       (b*num, d, s+2k)
    pe_q = depthwise_valid_xcorr(pe_k, w)            k=200 taps, same filter per channel
    qhat[b,nu,t] = sum_d pe_q[bn,d,t]      * q[b,d,t]
    khat[b,nu,t] = sum_d pe_k[bn,d,t+k//2] * k[b,d,t]

Kernel strategy (8 NeuronCores, data-parallel over the 128 bn rows; 16 rows/core):
  * Host pre-arranges noise into a time-partition-inner fp16 layout
    xf[bn][p, n, d] = noise[bn, d, 128n+p] so the conv becomes 3 PSUM-accumulated
    TensorE matmuls per output block with fixed Toeplitz weights
    W_s[p, m] = w[p + 128s - m] (shared across all channels/rows).
  * qhat: ScalarE copies conv PSUM -> SBUF fp16, VectorE multiplies by
    host-pre-transposed queries (fp16 2x mode) and reduces over d.
  * khat needs no conv: VectorE multiplies xf by a host-shifted/scaled keys
    layout (shift k//2=100 and 1/sqrt(num*d) baked in); GpSimd reduces over d
    with an fp32 add-tree (engine balancing: DVE is the bottleneck).
"""

import math
import numpy as np

_CACHE = {}


def _ensure_paths():
    try:
        import concourse  # noqa: F401
    except ImportError:
        import sys

        for p in ("/opt/trn_rl_repo", "/root/.axon_site/_ro/trn_rl_repo"):
            if p not in sys.path:
                sys.path.insert(0, p)


N_CORES = 8
B, D, L, K, NUM = 4, 64, 4096, 200, 32
NW = 34  # x windows of 128 loaded per row (covers t+j up to 4351)
NT = 32  # output time blocks of 128
NK = 33  # khat product blocks (u = t + 100 spans [0, 4224))
ROWS = 16  # bn rows per core


def _add_tree(eng, pool, acc_out, src, n_outer, bn, mybir):
    """Reduce src [128, n_outer, 64] over the last axis into acc_out [128, n_outer]
    using TT-adds (fp32 after level 1). Works on engines without X-axis reduce."""
    F32 = mybir.dt.float32
    a = pool.tile([128, n_outer, 32], F32, tag="treeA", name=f"treeA_{bn}")
    b = pool.tile([128, n_outer, 16], F32, tag="treeB", name=f"treeB_{bn}")
    eng.tensor_add(a[:], src[:, :, 0:32], src[:, :, 32:64])
    eng.tensor_add(b[:], a[:, :, 0:16], a[:, :, 16:32])
    eng.tensor_add(a[:, :, 0:8], b[:, :, 0:8], b[:, :, 8:16])
    eng.tensor_add(b[:, :, 0:4], a[:, :, 0:4], a[:, :, 4:8])
    eng.tensor_add(a[:, :, 8:10], b[:, :, 0:2], b[:, :, 2:4])
    eng.tensor_add(acc_out, a[:, :, 8], a[:, :, 9])


def build_module():
    """Build + compile the per-core Bass module (identical SPMD program)."""
    _ensure_paths()
    from contextlib import ExitStack

    import concourse.bacc as bacc
    import concourse.mybir as mybir
    import concourse.tile as tile

    F16 = mybir.dt.float16
    F32 = mybir.dt.float32
    X = mybir.AxisListType.X

    nc = bacc.Bacc(
        "TRN2", target_bir_lowering=False, debug=False, num_devices=N_CORES
    )

    xf_d = nc.dram_tensor("xf", [ROWS, 128, NW, D], F16, kind="ExternalInput").ap()
    wq_d = nc.dram_tensor("wq", [3, 128, 128], F16, kind="ExternalInput").ap()
    qt_d = nc.dram_tensor("qt", [128, NT, D], F16, kind="ExternalInput").ap()
    kf_d = nc.dram_tensor("kf", [128, NK, D], F16, kind="ExternalInput").ap()
    qo_d = nc.dram_tensor("qo", [128, ROWS, NT], F32, kind="ExternalOutput").ap()
    ko_d = nc.dram_tensor("ko", [128, ROWS, NK], F32, kind="ExternalOutput").ap()

    with tile.TileContext(nc) as tc, ExitStack() as ctx:
        wp = ctx.enter_context(tc.tile_pool(name="const", bufs=1))
        xp = ctx.enter_context(tc.tile_pool(name="x", bufs=4))
        pp = ctx.enter_context(tc.tile_pool(name="psum", bufs=4, space="PSUM"))
        cp = ctx.enter_context(tc.tile_pool(name="peq", bufs=3))
        qp = ctx.enter_context(tc.tile_pool(name="prodq", bufs=3))
        kpool = ctx.enter_context(tc.tile_pool(name="prodk", bufs=3))
        tp = ctx.enter_context(tc.tile_pool(name="tree", bufs=3))
        op = ctx.enter_context(tc.tile_pool(name="out", bufs=1))

        wts = []
        for s in range(3):
            t = wp.tile([128, 128], F16, tag=f"w{s}")
            nc.sync.dma_start(t[:], wq_d[s])
            wts.append(t)
        qt_t = wp.tile([128, NT, D], F16, tag="qt")
        nc.sync.dma_start(qt_t[:], qt_d[:])
        kf_t = wp.tile([128, NK, D], F16, tag="kf")
        nc.sync.dma_start(kf_t[:], kf_d[:])

        qacc = op.tile([128, ROWS, NT], F32, tag="qa")
        kacc = op.tile([128, ROWS, NK], F32, tag="ka")

        for bn in range(ROWS):
            xt = xp.tile([128, NW, D], F16, tag="xt", name=f"xt_{bn}")
            nc.sync.dma_start(xt[:], xf_d[bn])

            # ---- khat path: pure elementwise + gpsimd reduce tree
            pk = kpool.tile([128, NK, D], F16, tag="pk", name=f"pk_{bn}")
            nc.vector.tensor_mul(pk[:], xt[:, 0:NK, :], kf_t[:])
            _add_tree(nc.gpsimd, tp, kacc[:, bn, :], pk, NK, bn, mybir)

            # ---- qhat path: conv via 3 Toeplitz matmuls per 8-block group.
            # Two 2-bank PSUM halves per row so ACT/DVE drain half 0 while
            # PE still works on half 1.
            for h in range(2):
                ps = pp.tile([128, NT // 2, D], F32, tag="ps", name=f"ps_{bn}_{h}")
                for s in range(3):
                    for g in range(2 * h, 2 * h + 2):
                        nc.tensor.matmul(
                            ps[:, (g - 2 * h) * 8 : (g - 2 * h + 1) * 8, :],
                            wts[s][:],
                            xt[:, g * 8 + s : g * 8 + s + 8, :],
                            start=(s == 0),
                            stop=(s == 2),
                        )
                peq = cp.tile([128, NT // 2, D], F16, tag="peq", name=f"peq_{bn}_{h}")
                nc.scalar.copy(peq[:], ps[:])
                pq = qp.tile([128, NT // 2, D], F16, tag="pq", name=f"pq_{bn}_{h}")
                nc.vector.tensor_mul(
                    pq[:], peq[:], qt_t[:, h * (NT // 2) : (h + 1) * (NT // 2), :]
                )
                nc.vector.reduce_sum(
                    qacc[:, bn, h * (NT // 2) : (h + 1) * (NT // 2)], pq[:], axis=X
                )

        nc.sync.dma_start(qo_d[:], qacc[:])
        nc.sync.dma_start(ko_d[:], kacc[:])

    nc.compile()
    return nc


def _get_module():
    if "nc" not in _CACHE:
        _CACHE["nc"] = build_module()
    return _CACHE["nc"]


def make_in_maps(queries, keys, noise, conv_weight, num):
    """Host-side shard + re-layout (all cheap numpy ops)."""
    num = int(np.asarray(num))
    queries = np.asarray(queries, dtype=np.float32)
    keys = np.asarray(keys, dtype=np.float32)
    noise = np.asarray(noise, dtype=np.float32)
    w = np.asarray(conv_weight, dtype=np.float32)[0, 0, :]
    scale = 1.0 / math.sqrt(num * D)

    # Toeplitz weights (scale folded in): W_s[p, m] = w[p + 128s - m] * scale
    p = np.arange(128)[:, None]
    m = np.arange(128)[None, :]
    Wq = np.zeros((3, 128, 128), np.float32)
    for s in range(3):
        j = p + 128 * s - m
        mask = (j >= 0) & (j < K)
        Wq[s][mask] = w[j[mask]] * scale
    Wq16 = Wq.astype(np.float16)

    # xf[bn][p, n, d] = noise[bn, d, 128n + p]
    xf = (
        noise[:, :, : NW * 128]
        .reshape(B * NUM, D, NW, 128)
        .transpose(0, 3, 2, 1)
        .astype(np.float16)
    )
    # qt[b][p, tau, d] = queries[b, d, 128 tau + p]
    qt = queries.reshape(B, D, NT, 128).transpose(0, 3, 2, 1).astype(np.float16)
    # kf[b][p, n, d] = keys[b, d, 128n + p - 100] * scale (zero out of range)
    kp = np.zeros((B, D, NK * 128), np.float32)
    kp[:, :, K // 2 : K // 2 + L] = keys * scale
    kf = kp.reshape(B, D, NK, 128).transpose(0, 3, 2, 1).astype(np.float16)

    in_maps = []
    for c in range(N_CORES):
        b = c // 2
        in_maps.append(
            {
                "xf": np.ascontiguousarray(xf[ROWS * c : ROWS * (c + 1)]),
                "wq": Wq16,
                "qt": np.ascontiguousarray(qt[b]),
                "kf": np.ascontiguousarray(kf[b]),
            }
        )
    return in_maps


def assemble_outputs(results):
    qhat = np.empty((B * NUM, L), np.float32)
    khat = np.empty((B * NUM, L), np.float32)
    for c in range(N_CORES):
        qo = results[c]["qo"]  # [128, ROWS, NT]
        ko = results[c]["ko"]  # [128, ROWS, NK]
        qhat[ROWS * c : ROWS * (c + 1)] = qo.transpose(1, 2, 0).reshape(ROWS, L)
        kv = ko.transpose(1, 2, 0).reshape(ROWS, NK * 128)
        khat[ROWS * c : ROWS * (c + 1)] = kv[:, K // 2 : K // 2 + L]
    return (
        qhat.reshape(B, NUM, L),
        khat.reshape(B, NUM, L),
    )


def kernel(queries, keys, noise, conv_weight, num):
    _ensure_paths()
    from concourse import bass_utils

    in_maps = make_in_maps(queries, keys, noise, conv_weight, num)
    nc = _get_module()
    res = bass_utils.run_bass_kernel_spmd(nc, in_maps, core_ids=list(range(N_CORES)))
    return assemble_outputs(res.results)
